# revision 38
# baseline (speedup 1.0000x reference)
"""BiGAT (2-omic projection + GATv2 conv + ELU) as a distributed Bass/Tile
kernel for 8 Trainium2 NeuronCores.

Strategy (graph/data parallel):
  - Nodes permuted so core c owns a balanced set of 50 dst blocks (slot
    assignment load-balances per-block edge counts, host-side).
  - Phase A: per-core projection from HOST-TRANSPOSED fp16 features; slab
    DMAs balanced across the scalar+sync queues (concurrent DMA engines);
    h kept RESIDENT in SBUF; xl written row-layout to DRAM per group.
  - Phase B: ONE AllGather of the xl table (12.8 MB).  While it runs on
    the collective cores, the xr side (xr = h@Wr) is computed from the
    resident h and transposed straight into an SBUF table.
  - Phase C processes dst blocks in PAIRS (per-call fixed costs of the
    hardware gathers halve; elementwise passes cover 2 blocks per
    instruction):
      * two batched gathers of xl[src] rows per pair (lo/hi src halves),
      * xr[dst] expanded per-edge via fp8 one-hot matmul fused with an
        identity-add of xl -> s = xl[src]+xr[dst] in PSUM,
      * lrelu via Relu(0.8 s) on Act + DVE add of 0.2 xl (in-place),
      * att-dot: DVE mult (in-place) + pairwise halvings split across
        gpsimd/DVE + tensor_reduce,
      * exp on Act; broadcast-expand + messages ex*xl split DVE/gpsimd,
      * scatter-add via fp8 one-hot matmuls into per-block PSUM accs,
      * per-16-block epilogue chunks (softmax normalize + bias + ELU)
        interleaved with later pairs, output DMA on alternating queues.
  - One-hot matrices are host-precomputed fp8, one DMA per pair.
"""

import sys
import numpy as np

sys.path.insert(0, "/opt/trn_rl_repo")

P = 128
H, C = 4, 32
HC = H * C
NEG_SLOPE = 0.2
E_SHIFT = -6.1     # exp(e + shift): keeps ex, ex*xl in fp16 range.
                   # -1.6 extra vs the full-logit kernel: the dropped
                   # 0.2*att.xr[dst] term (max +1.79) is a per-dst softmax
                   # shift that cancels in alpha; the shift re-centers it.

# phase C op placement (engine balance tuning)
MV_DVE_FRAC = 0.625   # fraction of pair-tiles whose message-mult runs on DVE
EXP_DVE_FRAC = 0.0    # fraction of pair-tiles whose ex-expand runs on DVE
NEPI = 5              # epilogue chunks


def configure(cores=8, n1=25000, n2=25000, d1=2000, d2=500,
              n1pad=3200, n2pad=3200, ng=512):
    global CORES, N1, N2, D1, D2, N1PC, N2PC, N1PAD, N2PAD
    global NPC, NB, NTOT, NG, D1P, D2P, HALF
    CORES, N1, N2, D1, D2 = cores, n1, n2, d1, d2
    N1PC, N2PC = N1 // CORES, N2 // CORES
    N1PAD, N2PAD = n1pad, n2pad
    NPC = N1PAD + N2PAD
    NB = NPC // 128
    NTOT = CORES * NPC
    NG = ng
    D1P = ((d1 + 127) // 128) * 128
    D2P = ((d2 + 127) // 128) * 128
    HALF = NTOT // 2
    assert HALF <= 32768          # int16 gather indices per section
    assert NB % 2 == 0


configure()


# ---------------------------------------------------------------------------
# host-side prep
# ---------------------------------------------------------------------------

def _wrap_idx(arr):
    L = arr.shape[0]
    w = arr.reshape(L // 16, 16).T.astype(np.int16)
    return np.tile(w, (8, 1))


def prep_edges(edge_index):
    """Balanced slots, per-core gather indices, fp8 one-hots, (F_LO, F_HI).

    lo section: src owned by cores 0..3 (gathered-table idx < HALF).
    hi section: src owned by cores 4..7.
    """
    import ml_dtypes
    global SLOT_OF, CORE_OF
    src, dst = edge_index[0].astype(np.int64), edge_index[1].astype(np.int64)

    allN = np.arange(N1 + N2)
    is1 = allN < N1
    CORE_OF = np.where(is1, allN // N1PC, (allN - N1) // N2PC)

    lo_src = CORE_OF[src] < CORES // 2
    lodeg = np.zeros(N1 + N2, np.int64)
    hideg = np.zeros(N1 + N2, np.int64)
    np.add.at(lodeg, dst[lo_src], 1)
    np.add.at(hideg, dst[~lo_src], 1)

    # balance dst nodes across blocks within each (core, omic) group
    SLOT_OF = np.zeros(N1 + N2, np.int64)
    for c in range(CORES):
        for base, lo_n, hi_n, nb0 in ((0, c * N1PC, (c + 1) * N1PC,
                                       N1PAD // 128),
                                      (N1PAD, N1 + c * N2PC,
                                       N1 + (c + 1) * N2PC, N2PAD // 128)):
            nodes = np.arange(lo_n, hi_n)
            lod = lodeg[nodes]
            hid = hideg[nodes]
            order = np.argsort(-(np.maximum(lod, hid) * 4096 + lod + hid),
                               kind="stable")
            cap = -(-len(nodes) // nb0)
            blo = np.zeros(nb0, np.int64)
            bhi = np.zeros(nb0, np.int64)
            fill = np.zeros(nb0, np.int64)
            for idx in order:
                l, h = lod[idx], hid[idx]
                score = np.maximum(blo + l, bhi + h) * 4096 + blo + bhi
                score[fill >= cap] = np.iinfo(np.int64).max
                b = int(np.argmin(score))
                SLOT_OF[nodes[idx]] = base + b * 128 + fill[b]
                blo[b] += l
                bhi[b] += h
                fill[b] += 1

    sc, ss = CORE_OF[src], SLOT_OF[src]
    dc, ds = CORE_OF[dst], SLOT_OF[dst]
    lo = lo_src.astype(np.int64)
    gidx = np.where(lo == 1, sc * NPC + ss, (sc - CORES // 2) * NPC + ss)
    blk = ds // 128

    order = np.lexsort((gidx, 1 - lo, blk, dc))
    gidx, dc, ds, blk, lo = (a[order] for a in (gidx, dc, ds, blk, lo))

    key = dc * NB + blk
    n_lo = np.zeros(CORES * NB, np.int64)
    n_hi = np.zeros(CORES * NB, np.int64)
    np.add.at(n_lo, key, lo)
    np.add.at(n_hi, key, 1 - lo)
    F_LO = int(np.max((n_lo + 127) // 128))
    F_HI = int(np.max((n_hi + 127) // 128))
    F = F_LO + F_HI

    gi_lo = np.zeros((CORES, NB, F_LO * 128), np.int16)
    gi_hi = np.zeros((CORES, NB, F_HI * 128), np.int16)
    dmod = np.full((CORES, NB, F, 128), 999, np.int32)

    bounds = np.searchsorted(key, np.arange(CORES * NB + 1))
    for k in range(CORES * NB):
        c, b = divmod(k, NB)
        a0, a1 = bounds[k], bounds[k + 1]
        nl = int(n_lo[k]); nh = int(n_hi[k])
        mod = (ds[a0:a1] % 128).astype(np.int32)
        gi_lo[c, b, :nl] = gidx[a0:a0 + nl]
        gi_hi[c, b, :nh] = gidx[a0 + nl:a1]
        dm = dmod[c, b].reshape(-1)
        dm[:nl] = mod[:nl]
        dm[F_LO * 128:F_LO * 128 + nh] = mod[nl:]

    j = np.arange(128, dtype=np.int32)
    out = []
    for c in range(CORES):
        glo = np.concatenate([_wrap_idx(gi_lo[c, b]) for b in range(NB)],
                             axis=1)
        ghi = np.concatenate([_wrap_idx(gi_hi[c, b]) for b in range(NB)],
                             axis=1)
        dmc = dmod[c]
        oh1 = (dmc[:, :, :, None] == j).astype(ml_dtypes.float8_e4m3)
        oh1 = oh1.transpose(2, 0, 1, 3).reshape(128, NB, F * 128)
        ohT = (dmc[:, :, None, :] == j[:, None]).astype(ml_dtypes.float8_e4m3)
        ohT = ohT.transpose(2, 0, 1, 3).reshape(128, NB, F * 128)
        ohc = np.concatenate([oh1, ohT], axis=2)
        out.append(dict(
            gilo=glo, gihi=ghi,
            ohc=np.ascontiguousarray(ohc.reshape(128, NB * 2 * F * 128))))
    return out, F_LO, F_HI


def _groups(npad):
    out = []
    g0 = 0
    while g0 < npad:
        out.append((g0, min(NG, npad - g0)))
        g0 += NG
    return out


def _slab(xT, npad):
    D = xT.shape[0]
    nch = D // 128
    xv = xT.reshape(nch, 128, npad)
    slabs = [np.ascontiguousarray(
                xv[:, :, g0:g0 + ng].transpose(1, 0, 2).reshape(128, nch * ng))
             for g0, ng in _groups(npad)]
    return np.concatenate(slabs, axis=1)


def prep_shards(x_mrna, x_mirna):
    shards = []
    for c in range(CORES):
        xm = np.zeros((D1P, N1PAD), np.float16)
        nodes1 = np.arange(c * N1PC, (c + 1) * N1PC)
        xm[:D1, SLOT_OF[nodes1]] = x_mrna[nodes1].T.astype(np.float16)
        xr_ = np.zeros((D2P, N2PAD), np.float16)
        nodes2 = np.arange(N1 + c * N2PC, N1 + (c + 1) * N2PC)
        xr_[:D2, SLOT_OF[nodes2] - N1PAD] = x_mirna[nodes2 - N1].T.astype(
            np.float16)
        shards.append((_slab(xm, N1PAD), _slab(xr_, N2PAD)))
    return shards


# ---------------------------------------------------------------------------
# program builder
# ---------------------------------------------------------------------------

def build_program(F_LO, F_HI):
    import concourse.bass as bass
    import concourse.mybir as mybir
    import concourse.tile as tile
    from concourse import bacc
    from concourse.masks import make_identity

    dt = mybir.dt
    f32 = dt.float32
    f16 = dt.float16
    f8 = dt.float8e4
    Alu = mybir.AluOpType
    Act = mybir.ActivationFunctionType
    F = F_LO + F_HI
    NT = 2 * F                       # tiles per block pair

    nc = bacc.Bacc("TRN2", target_bir_lowering=False, debug=False,
                   num_devices=CORES)

    xmT = nc.dram_tensor("xmT", [128, (D1P // 128) * N1PAD], f16,
                         kind="ExternalInput")
    xmiT = nc.dram_tensor("xmiT", [128, (D2P // 128) * N2PAD], f16,
                          kind="ExternalInput")
    wp1 = nc.dram_tensor("wp1", [D1P, P], f16, kind="ExternalInput")
    bp1 = nc.dram_tensor("bp1", [P, 1], f32, kind="ExternalInput")
    wp2 = nc.dram_tensor("wp2", [D2P, P], f16, kind="ExternalInput")
    bp2 = nc.dram_tensor("bp2", [P, 1], f32, kind="ExternalInput")
    wl = nc.dram_tensor("wl", [P, HC], f16, kind="ExternalInput")
    wr = nc.dram_tensor("wr", [P, HC], f16, kind="ExternalInput")
    attb = nc.dram_tensor("attb", [128, 128], f16, kind="ExternalInput")
    bgat = nc.dram_tensor("bgat", [128, 128], f16, kind="ExternalInput")
    gilo = nc.dram_tensor("gilo", [128, NB * F_LO * 8], dt.int16,
                          kind="ExternalInput")
    gihi = nc.dram_tensor("gihi", [128, NB * F_HI * 8], dt.int16,
                          kind="ExternalInput")
    ohc = nc.dram_tensor("ohc", [128, NB * 2 * F * 128], f8,
                         kind="ExternalInput")
    outp = nc.dram_tensor("outp", [NPC, HC], f16, kind="ExternalOutput")

    xl_loc = nc.dram_tensor("xl_loc", [NPC, HC], f16)
    xl_full = nc.dram_tensor("xl_full", [NTOT, HC], f16,
                             addr_space="Shared" if CORES > 4 else "Local")

    nch1 = D1P // 128
    nch2 = D2P // 128

    # pair-tile index: [LO(b0) | LO(b1) | HI(b0) | HI(b1)]
    def pt_of(b2, t):
        if t < F_LO:
            return b2 * F_LO + t
        return 2 * F_LO + b2 * F_HI + (t - F_LO)

    def inv_pt(pt):
        if pt < 2 * F_LO:
            return pt // F_LO, pt % F_LO
        q = pt - 2 * F_LO
        return q // F_HI, F_LO + q % F_HI

    with tile.TileContext(nc, num_cores=CORES) as tc:
        with tc.tile_pool(name="const", bufs=1) as cst:

            identh = cst.tile([128, 128], f16)
            make_identity(nc, identh[:])

            attb_sb = cst.tile([128, 128], f16)
            nc.sync.dma_start(attb_sb[:], attb.ap())
            bgat_sb = cst.tile([128, 128], f16)
            nc.scalar.dma_start(bgat_sb[:], bgat.ap())
            eshift_sb = cst.tile([128, 1], f32)
            nc.vector.memset(eshift_sb[:], E_SHIFT)
            # phase-C-only tables: keep their DMAs off the feature queues
            gilo_sb = cst.tile([128, NB * F_LO * 8], dt.int16)
            nc.gpsimd.dma_start(gilo_sb[:], gilo.ap())
            gihi_sb = cst.tile([128, NB * F_HI * 8], dt.int16)
            nc.gpsimd.dma_start(gihi_sb[:], gihi.ap())

            xrk_all = cst.tile([128, NPC], f16)    # resident xr row table

            # phase A/B residents, released before phase C
            pha_ctx = tc.tile_pool(name="pha", bufs=1)
            pha = pha_ctx.__enter__()
            wp1_sb = pha.tile([128, nch1 * 128], f16)
            nc.sync.dma_start(
                wp1_sb[:].rearrange("d (i j) -> d i j", j=128),
                wp1.ap().rearrange("(i d) j -> d i j", d=128))
            wp2_sb = pha.tile([128, nch2 * 128], f16)
            nc.scalar.dma_start(
                wp2_sb[:].rearrange("d (i j) -> d i j", j=128),
                wp2.ap().rearrange("(i d) j -> d i j", d=128))
            bp1_sb = pha.tile([128, 1], f32)
            nc.sync.dma_start(bp1_sb[:], bp1.ap())
            bp2_sb = pha.tile([128, 1], f32)
            nc.scalar.dma_start(bp2_sb[:], bp2.ap())
            wl_sb = pha.tile([128, HC], f16)
            nc.sync.dma_start(wl_sb[:], wl.ap())
            wr_sb = pha.tile([128, HC], f16)
            nc.scalar.dma_start(wr_sb[:], wr.ap())
            hT_all = pha.tile([128, NPC], f16)     # resident projections

            # ---------------- phase A: projections + xl table --------------
            # jobs from both omics, biggest first (smallest job last
            # minimizes the serial tail before the collective); slab DMAs
            # greedily balanced across the two HWDGE queues.
            jobs = []
            goff = [0, 0]
            for sec, npad in ((0, N1PAD), (1, N2PAD)):
                nchs = (nch1, nch2)[sec]
                for g0, ng in _groups(npad):
                    jobs.append((sec, g0, ng, goff[sec]))
                    goff[sec] += nchs * ng
            jobs.sort(key=lambda j: -((nch1, nch2)[j[0]] * j[2]))
            qload = [0, 0]
            queues = (nc.scalar, nc.sync)

            pa_ctx = tc.tile_pool(name="pa", bufs=3)
            pa = pa_ctx.__enter__()
            pa_ps_ctx = tc.tile_pool(name="pa_ps", bufs=2, space="PSUM")
            pa_ps = pa_ps_ctx.__enter__()
            pa_tps_ctx = tc.tile_pool(name="pa_tps", bufs=3, space="PSUM")
            pa_tps = pa_tps_ctx.__enter__()
            for sec, g0, ng, xoff in jobs:
                nchs = (nch1, nch2)[sec]
                xdram = (xmT, xmiT)[sec]
                wp_sb = (wp1_sb, wp2_sb)[sec]
                bp_sb = (bp1_sb, bp2_sb)[sec]
                row0 = (0, N1PAD)[sec]
                qi = 0 if qload[0] <= qload[1] else 1
                qload[qi] += nchs * ng
                dq = queues[qi]
                nt = ng // 128
                hps = pa_ps.tile([128, NG], f32, tag="hps", bufs=3)
                xTg = pa.tile([128, nch1 * NG], f16, tag="xTg")
                dq.dma_start(
                    xTg[:, :nchs * ng],
                    xdram.ap()[:, xoff:xoff + nchs * ng])
                for i in range(nchs):
                    nc.tensor.matmul(
                        hps[:, :ng],
                        lhsT=wp_sb[:, i * 128:(i + 1) * 128],
                        rhs=xTg[:, i * ng:(i + 1) * ng],
                        start=(i == 0), stop=(i == nchs - 1))
                r0 = row0 + g0
                nc.vector.tensor_scalar(hT_all[:, r0:r0 + ng],
                                        hps[:, :ng],
                                        bp_sb[:, 0:1], None, op0=Alu.add)
                xps = pa_ps.tile([128, NG], f32, tag="xps")
                nc.tensor.matmul(xps[:, :ng], lhsT=wl_sb[:],
                                 rhs=hT_all[:, r0:r0 + ng],
                                 start=True, stop=True)
                xsb = pa.tile([128, NG], f16, tag="xsb")
                nc.vector.tensor_copy(xsb[:, :ng], xps[:, :ng])
                rsb = pa.tile([128, NG], f16, tag="rsb")
                for ti in range(nt):
                    tp = pa_tps.tile([128, 128], f16, tag="tp")
                    nc.tensor.transpose(
                        tp[:], xsb[:, ti * 128:(ti + 1) * 128], identh[:])
                    nc.vector.tensor_copy(
                        rsb[:, ti * 128:(ti + 1) * 128], tp[:])
                nc.gpsimd.dma_start(
                    xl_loc.ap()[r0:r0 + ng, :]
                    .rearrange("(t p) j -> p t j", p=128),
                    rsb[:, :ng].rearrange("p (t j) -> p t j", j=128))

            pa_tps_ctx.__exit__(None, None, None)
            pa_ps_ctx.__exit__(None, None, None)
            pa_ctx.__exit__(None, None, None)

            # ---------------- phase B: single AllGather ---------------------
            nc.gpsimd.collective_compute(
                "AllGather", Alu.bypass,
                ins=[xl_loc.ap()],
                outs=[xl_full.ap().rearrange("(a b) j -> a b j", a=128)],
                replica_groups=[list(range(CORES))])

            # ---- xr side from resident h: runs during the AllGather -------
            with tc.tile_pool(name="px", bufs=3) as px, \
                 tc.tile_pool(name="px_ps", bufs=2, space="PSUM") as px_ps, \
                 tc.tile_pool(name="px_tps", bufs=4, space="PSUM") as px_tps:
                for g0, ng in _groups(NPC):
                    nt = ng // 128
                    xps = px_ps.tile([128, NG], f32, tag="xps2")
                    nc.tensor.matmul(xps[:, :ng], lhsT=wr_sb[:],
                                     rhs=hT_all[:, g0:g0 + ng],
                                     start=True, stop=True)
                    xsb = px.tile([128, NG], f16, tag="xsb2")
                    nc.vector.tensor_copy(xsb[:, :ng], xps[:, :ng])
                    for ti in range(nt):
                        tp = px_tps.tile([128, 128], f16, tag="tp2")
                        nc.tensor.transpose(
                            tp[:], xsb[:, ti * 128:(ti + 1) * 128], identh[:])
                        nc.vector.tensor_copy(
                            xrk_all[:, g0 + ti * 128:g0 + (ti + 1) * 128],
                            tp[:])

            pha_ctx.__exit__(None, None, None)

            # ---------------- phase C: edge processing ---------------------
            NCK = 8                                   # sps tiles per chunk
            NPAIR = NB // 2
            MVD = int(round(NT * MV_DVE_FRAC))        # mv tiles on DVE
            EXD = int(round(NT * EXP_DVE_FRAC))       # expand tiles on DVE
            EPB = (NB + NEPI - 1) // NEPI             # blocks per epi chunk

            with tc.tile_pool(name="stgp", bufs=1) as stgp, \
                 tc.tile_pool(name="epi", bufs=2) as epi, \
                 tc.tile_pool(name="pc", bufs=2) as pc, \
                 tc.tile_pool(name="pc_sps", bufs=3, space="PSUM") as pc_sps, \
                 tc.tile_pool(name="pc_acc", bufs=2, space="PSUM") as pc_acc:
                stg = stgp.tile([128, NB * 132], f16, tag="stg")

                def epilogue(k):
                    b0, b1 = k * EPB, min((k + 1) * EPB, NB)
                    nb = b1 - b0
                    sv = stg[:, b0 * 132:b1 * 132].rearrange(
                        "p (b x) -> p b x", b=nb)
                    dn = epi.tile([128, EPB * 4], f32, tag="dn")
                    nc.vector.tensor_scalar(
                        dn[:, :nb * 4], sv[:, :, 128:132], 1e-16, None,
                        op0=Alu.add)
                    rc = epi.tile([128, EPB * 4], f32, tag="rc")
                    nc.vector.reciprocal(rc[:, :nb * 4], dn[:, :nb * 4])
                    # undo the 0.2 table scaling on the message numerators
                    nc.vector.tensor_scalar(rc[:, :nb * 4], rc[:, :nb * 4],
                                            1.0 / NEG_SLOPE, None,
                                            op0=Alu.mult)
                    o1t = epi.tile([128, EPB * 128], f16, tag="o1")
                    o2t = epi.tile([128, EPB * 128], f16, tag="o2")
                    rce = epi.tile([128, EPB * 128], f16, tag="rce")
                    eng = (nc.vector, nc.gpsimd)[k % 2]
                    if k == NEPI - 1:
                        eng = nc.vector
                    o1v = o1t[:, :nb * 128]
                    # materialize the reciprocal broadcast on Act so the
                    # normalize multiply keeps DVE 2x mode
                    nc.scalar.copy(
                        rce[:, :nb * 128].rearrange("p (b h c) -> p b h c",
                                                    b=nb, h=H),
                        rc[:, :nb * 4].rearrange("p (b h) -> p b h", b=nb)
                        .unsqueeze(3).broadcast_to([128, nb, H, C]))
                    eng.tensor_tensor(
                        o1v.rearrange("p (b x) -> p b x", b=nb),
                        sv[:, :, 0:128],
                        rce[:, :nb * 128].rearrange("p (b x) -> p b x", b=nb),
                        op=Alu.mult)
                    eng.tensor_tensor(
                        o1v.rearrange("p (b x) -> p b x", b=nb),
                        o1v.rearrange("p (b x) -> p b x", b=nb),
                        bgat_sb[:].unsqueeze(1).broadcast_to([128, nb, 128]),
                        op=Alu.add)
                    o2v = o2t[:, :nb * 128]
                    eng.tensor_scalar(o2v, o1v, -1.0, 1.0,
                                      op0=Alu.min, op1=Alu.add)
                    nc.scalar.activation(o2v, o2v, Act.Exp)
                    eng.tensor_scalar(o1v, o1v, -1.0, None, op0=Alu.max)
                    eng.tensor_add(o1v, o1v, o2v)
                    queues[k % 2].dma_start(
                        outp.ap()[b0 * 128:b1 * 128, :]
                        .rearrange("(b p) j -> p b j", p=128),
                        o1v.rearrange("p (b j) -> p b j", b=nb))

                epi_done = 0
                state = {}

                # 4-stage software pipeline, one pair per stage per round.
                # Emission order per round i: S0(i) S1(i-1) S2(i-2) S3(i-3)
                # keeps every in-order engine queue free of head-of-line
                # waits on the previous pair's chain (PE: acc(j) queued
                # after sps(j+1); Act: exp(j) after relu(j+1)).

                def s0_fetch(j):
                    xlb_t = pc.tile([128, NT * 128], f16, tag="xlb", bufs=5)
                    xlb = xlb_t[:]
                    xlb3 = xlb.rearrange("p (n x) -> p n x", n=NT)
                    nc.gpsimd.dma_gather(
                        out_ap=xlb3[:, 0:2 * F_LO, :],
                        in_ap=xl_full.ap()[0:HALF, :],
                        idxs_ap=gilo_sb[:, 2 * j * F_LO * 8:
                                        2 * (j + 1) * F_LO * 8],
                        num_idxs=2 * F_LO * 128, num_idxs_reg=2 * F_LO * 128,
                        elem_size=HC, single_packet=False)
                    nc.gpsimd.dma_gather(
                        out_ap=xlb3[:, 2 * F_LO:NT, :],
                        in_ap=xl_full.ap()[HALF:NTOT, :],
                        idxs_ap=gihi_sb[:, 2 * j * F_HI * 8:
                                        2 * (j + 1) * F_HI * 8],
                        num_idxs=2 * F_HI * 128, num_idxs_reg=2 * F_HI * 128,
                        elem_size=HC, single_packet=False)
                    ohc_t_t = pc.tile([128, 2 * NT * 128], f8, tag="ohct",
                                      bufs=5)
                    ohc_t = ohc_t_t[:]
                    nc.sync.dma_start(
                        ohc_t,
                        ohc.ap()[:, j * 2 * NT * 128:(j + 1) * 2 * NT * 128])
                    state[j] = dict(xlb=xlb, xlb3=xlb3, ohc=ohc_t)

                def s1_sps_relu(j):
                    st = state[j]
                    xlb, ohc_t = st["xlb"], st["ohc"]
                    # Tables hold 0.2*xl / 0.2*xr (0.2 folded into Wl/Wr;
                    # epilogue multiplies the softmax reciprocal by 5), so
                    # sps = 0.2*s.  The logit att.lrelu(s) splits as
                    #   att.Relu(0.8 s) + 0.2 att.xl[src] + 0.2 att.xr[dst]
                    # and the last term is a per-dst softmax shift that
                    # cancels in alpha -> dropped (E_SHIFT re-centers it).
                    r08_t = pc.tile([128, NT * 128], f16, tag="r08", bufs=3)
                    r08 = r08_t[:]
                    for h0 in range(0, NT, NCK):
                        h1 = min(h0 + NCK, NT)
                        nh = h1 - h0
                        sps = pc_sps.tile([128, NCK * 128], f32, tag="sps")
                        for pt in range(h0, h1):
                            b2, t = inv_pt(pt)
                            o = (pt - h0) * 128
                            nc.tensor.matmul(
                                sps[:, o:o + 128],
                                lhsT=ohc_t[:, (b2 * 2 * F + F + t) * 128:
                                           (b2 * 2 * F + F + t + 1) * 128],
                                rhs=xrk_all[:, (2 * j + b2) * 128:
                                            (2 * j + b2 + 1) * 128],
                                start=True, stop=False)
                            nc.tensor.matmul(
                                sps[:, o:o + 128],
                                lhsT=identh[:],
                                rhs=xlb[:, pt * 128:(pt + 1) * 128],
                                start=False, stop=True)
                        nc.scalar.activation(r08[:, h0 * 128:h1 * 128],
                                             sps[:, :nh * 128], Act.Relu,
                                             scale=(1.0 - NEG_SLOPE) /
                                                   NEG_SLOPE)
                    st["r08"] = r08

                def s2_chain(j):
                    st = state[j]
                    xlb, xlb3, r08 = st["xlb"], st["xlb3"], st["r08"]
                    # g = r08 + 0.2 xl[src]; pm = g * att  (both in-place)
                    nc.vector.tensor_add(r08, r08, xlb)
                    nc.vector.tensor_tensor(
                        r08.rearrange("p (n x) -> p n x", n=NT),
                        r08.rearrange("p (n x) -> p n x", n=NT),
                        attb_sb[:].unsqueeze(1).broadcast_to([128, NT, 128]),
                        op=Alu.mult)
                    pm4 = r08.rearrange("p (n h x) -> p n h x", n=NT, h=H)
                    tr1 = pc.tile([128, NT * 64], f16, tag="tr1")
                    t1 = tr1[:].rearrange("p (n h x) -> p n h x", n=NT, h=H)
                    nc.vector.tensor_tensor(t1, pm4[:, :, :, 0:16],
                                            pm4[:, :, :, 16:32], op=Alu.add)
                    tr2 = pc.tile([128, NT * 32], f16, tag="tr2")
                    t2 = tr2[:].rearrange("p (n h x) -> p n h x", n=NT, h=H)
                    nc.vector.tensor_tensor(t2, t1[:, :, :, 0:8],
                                            t1[:, :, :, 8:16], op=Alu.add)
                    tr3 = pc.tile([128, NT * 16], f16, tag="tr3")
                    t3 = tr3[:].rearrange("p (n h x) -> p n h x", n=NT, h=H)
                    nc.gpsimd.tensor_tensor(t3, t2[:, :, :, 0:4],
                                            t2[:, :, :, 4:8], op=Alu.add)
                    e4f = pc.tile([128, NT * 4], f32, tag="e4f")
                    nc.vector.tensor_reduce(
                        e4f[:].rearrange("p (n h) -> p n h", n=NT),
                        t3, axis=mybir.AxisListType.X, op=Alu.add)
                    msgex = pc.tile([128, NT * 132], f16, tag="msgex",
                                    bufs=4)
                    mv = msgex[:].rearrange("p (n x) -> p n x", n=NT, x=132)
                    nc.scalar.activation(
                        mv[:, :, 128:132],
                        e4f[:].rearrange("p (n h) -> p n h", n=NT), Act.Exp,
                        bias=eshift_sb[:, 0:1])
                    st["mv"] = mv

                def s2b_mv(j):
                    st = state[j]
                    xlb3, mv = st["xlb3"], st["mv"]
                    # messages ex*xl: gpsimd's cost is stride-agnostic, so
                    # the ex operand broadcasts inline (no expand pass)
                    mv4 = mv[:, :, 0:128].rearrange("p n (h x) -> p n h x",
                                                    h=H)
                    exb = mv[:, :, 128:132].unsqueeze(3)
                    nc.gpsimd.tensor_tensor(
                        mv4,
                        xlb3.rearrange("p n (h x) -> p n h x", h=H),
                        exb.broadcast_to([128, NT, H, C]),
                        op=Alu.mult)

                def s3_scatter(j):
                    st = state.pop(j)
                    ohc_t, mv = st["ohc"], st["mv"]
                    for b2 in range(2):
                        acc = pc_acc.tile([128, 132], f32, tag="acc")
                        for t in range(F):
                            pt = pt_of(b2, t)
                            nc.tensor.matmul(
                                acc[:],
                                lhsT=ohc_t[:, (b2 * 2 * F + t) * 128:
                                           (b2 * 2 * F + t + 1) * 128],
                                rhs=mv[:, pt, :],
                                start=(t == 0), stop=(t == F - 1))
                        b = 2 * j + b2
                        nc.scalar.copy(stg[:, b * 132:(b + 1) * 132], acc[:])

                for i in range(NPAIR + 4):
                    if i < NPAIR:
                        s0_fetch(i)
                    if 0 <= i - 1 < NPAIR:
                        s1_sps_relu(i - 1)
                    if 0 <= i - 2 < NPAIR:
                        s2_chain(i - 2)
                    if 0 <= i - 3 < NPAIR:
                        s2b_mv(i - 3)
                    if 0 <= i - 4 < NPAIR:
                        s3_scatter(i - 4)
                        # fire epilogue chunks as their stg blocks complete
                        while (epi_done + 1) * EPB <= 2 * (i - 3) \
                                and epi_done < NEPI:
                            epilogue(epi_done)
                            epi_done += 1
                while epi_done < NEPI:
                    epilogue(epi_done)
                    epi_done += 1

    nc.compile()
    return nc


# ---------------------------------------------------------------------------
# entry point
# ---------------------------------------------------------------------------

def _make_in_maps(inputs):
    x_mrna = np.asarray(inputs["x_mrna"], np.float32)
    x_mirna = np.asarray(inputs["x_mirna"], np.float32)
    att = np.asarray(inputs["att"], np.float32)
    edge_index = np.asarray(inputs["edge_index"])

    edge_arrays, F_LO, F_HI = prep_edges(edge_index)
    shards = prep_shards(x_mrna, x_mirna)

    att_flat = att.reshape(HC)
    attb = np.tile(att_flat[None, :], (128, 1)).astype(np.float16)
    bgatb = np.tile(np.asarray(inputs["b_gat"], np.float32)[None, :] - 1.0,
                    (128, 1)).astype(np.float16)

    wp1p = np.zeros((D1P, P), np.float16)
    wp1p[:D1] = np.asarray(inputs["Wp1"], np.float32).astype(np.float16)
    wp2p = np.zeros((D2P, P), np.float16)
    wp2p[:D2] = np.asarray(inputs["Wp2"], np.float32).astype(np.float16)
    common = dict(
        wp1=wp1p,
        bp1=np.asarray(inputs["bp1"], np.float32).reshape(P, 1),
        wp2=wp2p,
        bp2=np.asarray(inputs["bp2"], np.float32).reshape(P, 1),
        wl=(NEG_SLOPE * np.asarray(inputs["Wl"], np.float32))
        .astype(np.float16),
        wr=(NEG_SLOPE * np.asarray(inputs["Wr"], np.float32))
        .astype(np.float16),
        attb=attb, bgat=bgatb)

    in_maps = []
    for c in range(CORES):
        xmc, xrc = shards[c]
        m = dict(common)
        m.update(xmT=xmc, xmiT=xrc, **edge_arrays[c])
        in_maps.append(m)
    return in_maps, F_LO, F_HI


def _assemble(results):
    out = np.empty((N1 + N2, HC), np.float32)
    for c in range(CORES):
        o = np.asarray(results[c]["outp"], np.float32)
        nodes1 = np.arange(c * N1PC, (c + 1) * N1PC)
        out[nodes1] = o[SLOT_OF[nodes1]]
        nodes2 = np.arange(N1 + c * N2PC, N1 + (c + 1) * N2PC)
        out[nodes2] = o[SLOT_OF[nodes2]]
    return out


def kernel(**inputs):
    from concourse.bass_utils import run_bass_kernel_spmd

    in_maps, F_LO, F_HI = _make_in_maps(inputs)
    nc = build_program(F_LO, F_HI)
    res = run_bass_kernel_spmd(nc, in_maps, list(range(CORES)))
    return _assemble(res.results)


if __name__ == "__main__":
    rng = np.random.default_rng(0)
    ei = rng.integers(0, N1 + N2, size=(2, 800000), dtype=np.int32)
    arrs, flo, fhi = prep_edges(ei)
    print("F_LO", flo, "F_HI", fhi)
